# revision 18
# baseline (speedup 1.0000x reference)
"""Trainium2 Bass kernel for nn_BackgroundFirstSourceFieldEEG (dense attention
with Gaussian-distance + low-rank leadfield softmax bias).

Strategy
--------
Data-parallel over batch*n_steps: the 4*32 = 128 (b, n) attention steps are
split 16-per-core across 8 NeuronCores (each core lands inside a single b, so
positional-bias factors are per-core constants).

All matmuls run in bf16 (fp32 PSUM accumulation). Measured on this part, a
back-to-back N=512 matmul stream sustains ~280-315 ns/MM regardless of
f32r/bf16 (self-loading weight stream dominates the gap to the 213 ns
roofline), so the win over the f32r baseline comes from cutting the MM count
per step from 76 to 64:

  - The softmax positional bias (Gaussian distance + low-rank leadfield) is
    precomputed on the host as EB = exp(bias) [K, Q] fp32->bf16 (constant per
    core, key_mask folded in as EB=0) and applied as one DVE multiply on the
    exp'd logits per head - this removes the 8 per-head bias matmuls and the
    whole Dekker-split machinery the f32r version needed.
  - bo (+ Wo @ bv) is added via the DVE copy-back against a host-broadcast
    [128, D] table - removes 4 ones-row matmuls per step.

Per step: 16 q-proj + 4 k-proj (amortized) + 4 v-proj + 8 QK + 8 AV(pair)
+ 8 sel-sum + 16 o-proj = 64 MMs.

Activations stay transposed (feature dim on partitions) end-to-end:
  qT[d,i] = WqT.T @ queryT  (bias+scale fused in the ACT copy-back)
  kT[d,j] = WkT.T @ kvT     (+bk, batched 4 steps -> 512-wide)
  v[j,d]  = kvT.T @ WvT     (bv folded into bo on host: sum_j attn = 1)
  logitsT[j,i] = kT_h.T @ qT_h      (64-row contraction per head)
  ex = exp(logitsT) * EB            (ACT exp, DVE mul)
  oT[d,i] = v_h.T @ ex_h            (head pairs packed into one PSUM bank)
  sums via select-matrix matmuls, reciprocal+normalize on DVE
  out[i,do] = oT.T @ WoT            (+bo via DVE add of broadcast table)
"""

import sys

for _p in ("/opt/trn_rl_repo", "/root/.axon_site/_ro/trn_rl_repo"):
    if _p not in sys.path:
        sys.path.insert(0, _p)

import numpy as np

import bass_rust
import concourse.bass as bass
import concourse.mybir as mybir
import concourse.tile as tile
from concourse.bass_utils import run_bass_kernel_spmd

F32 = mybir.dt.float32
BF16 = mybir.dt.bfloat16
NPBF16 = mybir.dt.np(BF16)
ACT_IDENT = mybir.ActivationFunctionType.Identity
ACT_EXP = mybir.ActivationFunctionType.Exp
ACT_RECIP = mybir.ActivationFunctionType.Reciprocal
MUL = mybir.AluOpType.mult
ADD = mybir.AluOpType.add

B, N, Q, K, D = 4, 32, 512, 128, 512
H, HD = 8, 64
LOW_RANK = 8
SCALE = HD ** -0.5
SIGMA = 0.05
SIG = 1.0 / (2.0 * max(SIGMA * SIGMA, 1e-6))
CORES = 8
S = (B * N) // CORES  # steps per core
P = 128


# ---------------------------------------------------------------- wait split
def _split_waits(nc, cap_mm=1, cap_other=1):
    """walrus in this container rejects instructions with more than ~1 sync
    wait (self-loading matmuls) / few (ctrl). Move excess waits onto
    same-engine NoOps inserted right before the instruction."""
    n = 0
    for fn in nc.m.functions:
        for bb in fn.blocks:
            insts = bb.instructions  # live list
            i = 0
            while i < len(insts):
                inst = insts[i]
                si = inst.sync_info
                if si is None:
                    i += 1
                    continue
                cap = cap_mm if isinstance(inst, mybir.InstMatmult) else cap_other
                waits = list(si.on_wait)
                if len(waits) <= cap:
                    i += 1
                    continue
                keep, extra = waits[-cap:], waits[:-cap]
                for k, w in enumerate(extra):
                    n += 1
                    nop = mybir.InstNoOp(name=f"wsplit_{n}", ins=[], outs=[])
                    nop.engine = inst.engine
                    nop.sync_info = bass_rust.SyncInfo(on_wait=[w], on_update=[])
                    insts.insert(i + k, nop)
                inst.sync_info = bass_rust.SyncInfo(
                    on_wait=keep, on_update=list(si.on_update)
                )
                i += len(extra) + 1


# ---------------------------------------------------------------- device IR
def build_nc(n_steps=S, split_waits=True, repeat=1, stages=("dma", "compute")):
    groups = (n_steps + 3) // 4
    nc = bass.Bass("TRN2", target_bir_lowering=False, debug=False, num_devices=CORES)

    def din(name, shape, dt=BF16):
        return nc.dram_tensor(name, list(shape), dt, kind="ExternalInput").ap()

    qT_d = din("qT", (n_steps, D, Q))
    kvT_d = din("kvT", (groups, D, 4 * K))
    wq_d = din("wq", (D, D))
    wk_d = din("wk", (D, D))
    wv_d = din("wv", (D, D))
    wo_d = din("wo", (D, D))
    bqs_d = din("bqs", (D,), F32)
    bk_d = din("bk", (D,), F32)
    bo_d = din("bob", (P, D), F32)
    eb_d = din("EB", (K, Q))
    sel0_d = din("sel0", (K, P))
    sel1_d = din("sel1", (K, P))
    vzero_d = din("vzero", (P, 4, 2, P))
    out_d = nc.dram_tensor("out", [n_steps, Q, D], F32, kind="ExternalOutput").ap()

    from contextlib import ExitStack

    with tile.TileContext(nc) as tc, nc.allow_low_precision(
        reason="bf16 matmuls: tolerance is 2e-2, bf16 lands ~5e-3"
    ), ExitStack() as stack:
        ec = stack.enter_context
        cst = ec(tc.tile_pool(name="cst", bufs=1))
        qin_p = ec(tc.tile_pool(name="qin", bufs=2))
        kv_p = ec(tc.tile_pool(name="kv", bufs=2))
        qt_p = ec(tc.tile_pool(name="qt", bufs=8))
        kt_p = ec(tc.tile_pool(name="kt", bufs=8))
        ex_p = ec(tc.tile_pool(name="ex", bufs=6))
        e2_p = ec(tc.tile_pool(name="e2", bufs=10))
        rb_p = ec(tc.tile_pool(name="rb", bufs=4))
        ot_p = ec(tc.tile_pool(name="ot", bufs=8))
        oo_p = ec(tc.tile_pool(name="oo", bufs=3))
        pp = ec(tc.tile_pool(name="pp", bufs=2, space="PSUM"))
        pl = ec(tc.tile_pool(name="pl", bufs=2, space="PSUM"))
        po = ec(tc.tile_pool(name="po", bufs=2, space="PSUM"))
        pm = ec(tc.tile_pool(name="pm", bufs=2, space="PSUM"))
        if True:
            # ---- constants
            wq_sb = cst.tile([P, 4, D], BF16)
            wk_sb = cst.tile([P, 4, D], BF16)
            wv_sb = cst.tile([P, 4, D], BF16)
            wo_sb = cst.tile([P, 4, D], BF16)
            for w_sb, w_d in ((wq_sb, wq_d), (wk_sb, wk_d), (wv_sb, wv_d), (wo_sb, wo_d)):
                nc.sync.dma_start(w_sb, w_d.rearrange("(ct p) d -> p ct d", p=P))
            bqs_sb = cst.tile([P, 4], F32)
            bk_sb = cst.tile([P, 4], F32)
            nc.sync.dma_start(bqs_sb, bqs_d.rearrange("(dt p) -> p dt", p=P))
            nc.sync.dma_start(bk_sb, bk_d.rearrange("(dt p) -> p dt", p=P))
            bo_sb = cst.tile([P, D], F32)
            nc.sync.dma_start(bo_sb, bo_d)
            eb_sb = cst.tile([K, Q], BF16)
            nc.sync.dma_start(eb_sb, eb_d)
            sel0_sb = cst.tile([K, P], BF16)
            sel1_sb = cst.tile([K, P], BF16)
            nc.sync.dma_start(sel0_sb, sel0_d)
            nc.sync.dma_start(sel1_sb, sel1_d)
            # persistent double-buffered zero-padded v tiles (zero halves DMA'd
            # once; per-step DVE copies only touch the v halves)
            vp_bufs = []
            for i in range(2):
                vb = cst.tile([P, 4, 2, P], BF16, tag=f"vp{i}")
                nc.sync.dma_start(vb, vzero_d)
                vp_bufs.append(vb)

            from contextlib import nullcontext

            loop_cm = tc.For_i(0, repeat, 1) if repeat > 1 else nullcontext()
            with loop_cm:
                body_steps(
                    nc, n_steps,
                    kv_p, qin_p, qt_p, kt_p, ex_p, e2_p, rb_p, ot_p, oo_p,
                    pp, pl, po, pm,
                    wq_sb, wk_sb, wv_sb, wo_sb, bqs_sb, bk_sb, bo_sb,
                    eb_sb, sel0_sb, sel1_sb, vp_bufs,
                    qT_d, kvT_d, out_d, stages,
                )

    if split_waits:
        _split_waits(nc)
    return nc


def body_steps(
    nc, n_steps,
    kv_p, qin_p, qt_p, kt_p, ex_p, e2_p, rb_p, ot_p, oo_p,
    pp, pl, po, pm,
    wq_sb, wk_sb, wv_sb, wo_sb, bqs_sb, bk_sb, bo_sb,
    eb_sb, sel0_sb, sel1_sb, vp_bufs,
    qT_d, kvT_d, out_d, stages=("dma", "compute"),
):
    do_dma = "dma" in stages
    do_compute = "compute" in stages
    groups = (n_steps + 3) // 4

    if not do_compute:
        # DMA-only stage for bisect timing
        for s in range(n_steps):
            g, s4 = divmod(s, 4)
            if do_dma:
                if s4 == 0:
                    kv_sb = kv_p.tile([P, 4, 4 * K], BF16, tag="kv")
                    nc.sync.dma_start(
                        kv_sb, kvT_d[g].rearrange("(ct p) x -> p ct x", p=P)
                    )
                qin_sb = qin_p.tile([P, 4, Q], BF16, tag="qin")
                nc.sync.dma_start(
                    qin_sb, qT_d[s].rearrange("(ct p) i -> p ct i", p=P)
                )
                oo_t = oo_p.tile([P, 4, D], F32, tag="oo")
                nc.vector.tensor_copy(oo_t[:, 0:2, :], qin_sb[:, 0:2, :])
                nc.vector.tensor_copy(oo_t[:, 2:4, :], qin_sb[:, 2:4, :])
                nc.sync.dma_start(
                    out_d[s].rearrange("(it p) d -> p it d", p=P), oo_t
                )
        return

    # ---------------- pipeline state
    kv_tiles = {}
    qin_tiles = {}
    qts = {}   # step -> [qt tile per dt]
    kts = {}   # group -> [kt tile per dt]
    e2s = {}   # step -> [e2 tile per head]
    ots = {}   # step -> [ot tile per pair]
    oos = {}   # step -> oo tile

    def dma_kv(g):
        if not do_dma or g >= groups or g in kv_tiles:
            return
        t = kv_p.tile([P, 4, 4 * K], BF16, tag="kv")
        nc.sync.dma_start(t, kvT_d[g].rearrange("(ct p) x -> p ct x", p=P))
        kv_tiles[g] = t

    def dma_qin(s):
        if not do_dma or s >= n_steps or s in qin_tiles:
            return
        t = qin_p.tile([P, 4, Q], BF16, tag="qin")
        nc.sync.dma_start(t, qT_d[s].rearrange("(ct p) i -> p ct i", p=P))
        qin_tiles[s] = t

    def kproj_dt(g, dt):
        kv_sb = kv_tiles[g] if do_dma else wk_sb
        ps_k = pp.tile([P, Q], F32, tag="pp")
        for ct in range(4):
            nc.tensor.matmul(
                ps_k,
                wk_sb[:, ct, dt * P : (dt + 1) * P],
                kv_sb[:, ct, :],
                start=(ct == 0),
                stop=(ct == 3),
            )
        kt_t = kt_p.tile([P, 4, K], BF16, tag="kt")
        nc.scalar.activation(
            kt_t.rearrange("p s j -> p (s j)"), ps_k, ACT_IDENT,
            bias=bk_sb[:, dt : dt + 1],
        )
        kts.setdefault(g, []).append(kt_t)

    def qproj_dt(s, dt):
        qin_sb = qin_tiles[s] if do_dma else wq_sb
        ps_q = pp.tile([P, Q], F32, tag="pp")
        for ct in range(4):
            nc.tensor.matmul(
                ps_q,
                wq_sb[:, ct, dt * P : (dt + 1) * P],
                qin_sb[:, ct, :],
                start=(ct == 0),
                stop=(ct == 3),
            )
        qt_t = qt_p.tile([P, Q], BF16, tag="qt")
        nc.scalar.activation(
            qt_t, ps_q, ACT_IDENT, bias=bqs_sb[:, dt : dt + 1], scale=SCALE
        )
        qts.setdefault(s, []).append(qt_t)

    def vproj(s):
        g, s4 = divmod(s, 4)
        kv_sb = kv_tiles[g] if do_dma else wk_sb
        ps_v = pp.tile([P, D], F32, tag="pp")
        for ct in range(4):
            nc.tensor.matmul(
                ps_v,
                kv_sb[:, ct, s4 * K : (s4 + 1) * K],
                wv_sb[:, ct, :],
                start=(ct == 0),
                stop=(ct == 3),
            )
        # zero-padded per-head-pair lhsT tiles [j, dt, parity, 128]: even head
        # occupies columns 0:64, odd head 64:128, so the AV matmul pair lands
        # both heads in one PSUM bank at partition base 0.
        vp = vp_bufs[s % 2]
        psv4 = ps_v.rearrange("p (a b c) -> p a b c", b=2, c=HD)
        nc.vector.tensor_copy(vp[:, :, 0, 0:64], psv4[:, :, 0, :])
        nc.vector.tensor_copy(vp[:, :, 1, 64:128], psv4[:, :, 1, :])

    def proj_chunks(s):
        """Projection work for step s, split into filler-sized chunks."""
        if s >= n_steps:
            return []
        g, s4 = divmod(s, 4)
        out = []
        if s4 == 0:
            out.append(lambda: (kproj_dt(g, 0), kproj_dt(g, 1)))
            out.append(lambda: (kproj_dt(g, 2), kproj_dt(g, 3)))
        out.append(lambda: (qproj_dt(s, 0), qproj_dt(s, 1)))
        out.append(lambda: (qproj_dt(s, 2), qproj_dt(s, 3)))
        out.append(lambda: vproj(s))
        return out

    def qk(s, h):
        """One head's logits matmul + exp (ACT) + positional-bias multiply
        (DVE, precomputed EB table; also applies key_mask)."""
        g, s4 = divmod(s, 4)
        hb = 64 * (h % 2)
        dt = h // 2
        ps_l = pl.tile([P, Q], F32, tag="pl")
        nc.tensor.matmul(
            ps_l,
            kts[g][dt][hb : hb + 64, s4, :],
            qts[s][dt][hb : hb + 64, :],
            start=True,
            stop=True,
        )
        ex_t = ex_p.tile([P, Q], BF16, tag="ex")
        nc.scalar.activation(ex_t, ps_l, ACT_EXP)
        e2_t = e2_p.tile([P, Q], BF16, tag="e2")
        nc.vector.tensor_tensor(e2_t, ex_t, eb_sb, MUL)
        e2s.setdefault(s, []).append(e2_t)

    def pair(s, p):
        """AV matmuls + softmax-sum matmuls + normalize for head pair p."""
        vp = vp_bufs[s % 2]
        ps_pair = po.tile([P, Q], F32, tag="po")
        ps_sum = pm.tile([P, Q], F32, tag="pm")
        for par in range(2):
            nc.tensor.matmul(
                ps_pair, vp[:, p, par, :], e2s[s][2 * p + par],
                start=(par == 0), stop=(par == 1),
            )
            nc.tensor.matmul(
                ps_sum, sel0_sb if par == 0 else sel1_sb, e2s[s][2 * p + par],
                start=(par == 0), stop=(par == 1),
            )
        rb_t = rb_p.tile([P, Q], F32, tag="rb")
        nc.vector.reciprocal(rb_t, ps_sum)
        ot_t = ot_p.tile([P, Q], BF16, tag="ot")
        nc.vector.tensor_tensor(ot_t, ps_pair, rb_t, MUL)
        ots.setdefault(s, []).append(ot_t)

    def oproj_it(s, it):
        if s < 0 or s >= n_steps:
            return
        if s not in oos:
            oos[s] = oo_p.tile([P, 4, D], F32, tag="oo", name=f"oo_{s}")
        ps_f = pp.tile([P, D], F32, tag="pp")
        for dt in range(4):
            nc.tensor.matmul(
                ps_f,
                ots[s][dt][:, it * P : (it + 1) * P],
                wo_sb[:, dt, :],
                start=(dt == 0),
                stop=(dt == 3),
            )
        nc.vector.tensor_tensor(oos[s][:, it, :], ps_f, bo_sb, ADD)

    def dma_out(s):
        if s < 0 or s >= n_steps or not do_dma:
            return
        nc.sync.dma_start(
            out_d[s].rearrange("(it p) d -> p it d", p=P), oos[s]
        )

    # ---------------- pipelined emission
    # step s: QK section (o-proj of s-1 interleaved between QK head pairs),
    # then AV pairs (projections of s+1 interleaved between pairs).
    dma_kv(0)
    dma_qin(0)
    for c in proj_chunks(0):
        c()
    for s in range(n_steps):
        g, s4 = divmod(s, 4)
        # input prefetch ~1 step ahead of the projections that consume them
        dma_qin(s + 1)
        if s4 == 1:
            dma_kv(g + 1)
        qk(s, 0)
        qk(s, 1)
        for blk in range(3):
            if s > 0:
                oproj_it(s - 1, blk)
            qk(s, 2 + 2 * blk)
            qk(s, 3 + 2 * blk)
        if s > 0:
            oproj_it(s - 1, 3)
            dma_out(s - 1)
        nxt = proj_chunks(s + 1)
        pair(s, 0)
        pair(s, 1)
        if nxt:
            nxt[0]()
        pair(s, 2)
        if len(nxt) > 1:
            nxt[1]()
        pair(s, 3)
        for c in nxt[2:]:
            c()
        # drop refs no longer needed
        e2s.pop(s, None)
        if s >= 2:
            qts.pop(s - 2, None)
            ots.pop(s - 2, None)
            oos.pop(s - 2, None)
    for it in range(4):
        oproj_it(n_steps - 1, it)
    dma_out(n_steps - 1)


# ---------------------------------------------------------------- host prep
def make_in_maps(inputs, n_steps=S, cores=CORES):
    q_in = np.ascontiguousarray(np.asarray(inputs["query"], dtype=np.float32))
    kv_in = np.ascontiguousarray(np.asarray(inputs["key_value"], dtype=np.float32))
    qp = np.asarray(inputs["query_pos"], dtype=np.float32)
    kp = np.asarray(inputs["key_pos"], dtype=np.float32)
    mask = np.asarray(inputs["key_mask"])
    Wq = np.asarray(inputs["Wq"], dtype=np.float32)
    Wk = np.asarray(inputs["Wk"], dtype=np.float32)
    Wv = np.asarray(inputs["Wv"], dtype=np.float32)
    Wo = np.asarray(inputs["Wo"], dtype=np.float32)
    bq = np.asarray(inputs["bq"], dtype=np.float32)
    bk = np.asarray(inputs["bk"], dtype=np.float32)
    bv = np.asarray(inputs["bv"], dtype=np.float32)
    bo = np.asarray(inputs["bo"], dtype=np.float32)
    Wqb = np.asarray(inputs["Wqb"], dtype=np.float32)
    Wkb = np.asarray(inputs["Wkb"], dtype=np.float32)

    groups = (n_steps + 3) // 4
    bo2 = (
        bo.astype(np.float64) + Wo.astype(np.float64) @ bv.astype(np.float64)
    ).astype(np.float32)
    shared = {
        "wq": np.ascontiguousarray(Wq.T).astype(NPBF16),
        "wk": np.ascontiguousarray(Wk.T).astype(NPBF16),
        "wv": np.ascontiguousarray(Wv.T).astype(NPBF16),
        "wo": np.ascontiguousarray(Wo.T).astype(NPBF16),
        "bqs": (bq * SCALE).astype(np.float32),
        "bk": bk,
        "bob": np.broadcast_to(bo2, (P, D)).copy(),
        "vzero": np.zeros((P, 4, 2, P), NPBF16),
        "sel0": np.concatenate(
            [np.ones((K, 64), NPBF16), np.zeros((K, 64), NPBF16)], axis=1
        ),
        "sel1": np.concatenate(
            [np.zeros((K, 64), NPBF16), np.ones((K, 64), NPBF16)], axis=1
        ),
    }

    # EB[j, i] = exp(dist_bias + lead_bias) with key_mask folded in (fp64 on
    # the host, O(B*K*Q) tiny)
    eb_per_b = {}
    for b in range(B):
        kp64 = kp[b].astype(np.float64)
        qp64 = qp[b].astype(np.float64)
        d2 = ((kp64[:, None, :] - qp64[None, :, :]) ** 2).sum(-1)  # [K, Q]
        lead = (
            (kp64 @ Wkb.T.astype(np.float64)) @ (qp64 @ Wqb.T.astype(np.float64)).T
        ) / np.sqrt(float(LOW_RANK))
        bias = -d2 * SIG + lead
        eb = np.exp(bias)
        eb[~mask[b], :] = 0.0
        eb_per_b[b] = eb.astype(np.float32).astype(NPBF16)

    steps_per_b = N  # 32
    in_maps = []
    for c in range(cores):
        start = c * n_steps
        b = start // steps_per_b
        n0 = start % steps_per_b
        qT = np.ascontiguousarray(
            q_in[b, n0 : n0 + n_steps].transpose(0, 2, 1)
        ).astype(NPBF16)  # [S, D, Q]
        kvT = np.ascontiguousarray(
            kv_in[b, n0 : n0 + n_steps]
            .reshape(groups, 4, K, D)
            .transpose(0, 3, 1, 2)
            .reshape(groups, D, 4 * K)
        ).astype(NPBF16)
        in_maps.append({"qT": qT, "kvT": kvT, "EB": eb_per_b[b], **shared})
    return in_maps


_NC_CACHE = {}


def kernel(**inputs) -> np.ndarray:
    if S not in _NC_CACHE:
        _NC_CACHE[S] = build_nc(S)
    nc = _NC_CACHE[S]
    in_maps = make_in_maps(inputs)
    res = run_bass_kernel_spmd(nc, in_maps, core_ids=list(range(CORES)))
    out = np.empty((B, N, Q, D), np.float32)
    for c in range(CORES):
        start = c * S
        b = start // N
        n0 = start % N
        out[b, n0 : n0 + S] = res.results[c]["out"]
    return out


# revision 20
# speedup vs baseline: 1.0216x; 1.0216x over previous
"""Trainium2 Bass kernel for nn_BackgroundFirstSourceFieldEEG (dense attention
with Gaussian-distance + low-rank leadfield softmax bias).

Strategy
--------
Data-parallel over batch*n_steps: the 4*32 = 128 (b, n) attention steps are
split 16-per-core across 8 NeuronCores (each core lands inside a single b, so
positional-bias factors are per-core constants).

All matmuls run in bf16 (fp32 PSUM accumulation). Measured on this part, a
back-to-back N=512 matmul stream sustains ~280-315 ns/MM regardless of
f32r/bf16 (self-loading weight stream dominates the gap to the 213 ns
roofline), so the win over the f32r baseline comes from cutting the MM count
per step from 76 to 64:

  - The softmax positional bias (Gaussian distance + low-rank leadfield) is
    precomputed on the host as EB = exp(bias) [K, Q] fp32->bf16 (constant per
    core, key_mask folded in as EB=0) and applied as one DVE multiply on the
    exp'd logits per head - this removes the 8 per-head bias matmuls and the
    whole Dekker-split machinery the f32r version needed.
  - bo (+ Wo @ bv) is added via the DVE copy-back against a host-broadcast
    [128, D] table - removes 4 ones-row matmuls per step.

Per step: 16 q-proj + 4 k-proj (amortized) + 4 v-proj + 8 QK + 8 AV(pair)
+ 8 sel-sum + 16 o-proj = 64 MMs.

Activations stay transposed (feature dim on partitions) end-to-end:
  qT[d,i] = WqT.T @ queryT  (bias+scale fused in the ACT copy-back)
  kT[d,j] = WkT.T @ kvT     (+bk, batched 4 steps -> 512-wide)
  v[j,d]  = kvT.T @ WvT     (bv folded into bo on host: sum_j attn = 1)
  logitsT[j,i] = kT_h.T @ qT_h      (64-row contraction per head)
  ex = exp(logitsT) * EB            (ACT exp, DVE mul)
  oT[d,i] = v_h.T @ ex_h            (head pairs packed into one PSUM bank)
  sums via select-matrix matmuls, reciprocal+normalize on DVE
  out[i,do] = oT.T @ WoT            (+bo via DVE add of broadcast table)
"""

import sys

for _p in ("/opt/trn_rl_repo", "/root/.axon_site/_ro/trn_rl_repo"):
    if _p not in sys.path:
        sys.path.insert(0, _p)

import numpy as np

import bass_rust
import concourse.bass as bass
import concourse.mybir as mybir
import concourse.tile as tile
from concourse.bass_utils import run_bass_kernel_spmd

F32 = mybir.dt.float32
BF16 = mybir.dt.bfloat16
NPBF16 = mybir.dt.np(BF16)
ACT_IDENT = mybir.ActivationFunctionType.Identity
ACT_EXP = mybir.ActivationFunctionType.Exp
ACT_RECIP = mybir.ActivationFunctionType.Reciprocal
MUL = mybir.AluOpType.mult
ADD = mybir.AluOpType.add

B, N, Q, K, D = 4, 32, 512, 128, 512
H, HD = 8, 64
LOW_RANK = 8
SCALE = HD ** -0.5
SIGMA = 0.05
SIG = 1.0 / (2.0 * max(SIGMA * SIGMA, 1e-6))
CORES = 8
S = (B * N) // CORES  # steps per core
P = 128


# ---------------------------------------------------------------- wait split
def _split_waits(nc, cap_mm=1, cap_other=1):
    """walrus in this container rejects instructions with more than ~1 sync
    wait (self-loading matmuls) / few (ctrl). Move excess waits onto
    same-engine NoOps inserted right before the instruction."""
    n = 0
    for fn in nc.m.functions:
        for bb in fn.blocks:
            insts = bb.instructions  # live list
            i = 0
            while i < len(insts):
                inst = insts[i]
                si = inst.sync_info
                if si is None:
                    i += 1
                    continue
                cap = cap_mm if isinstance(inst, mybir.InstMatmult) else cap_other
                waits = list(si.on_wait)
                if len(waits) <= cap:
                    i += 1
                    continue
                keep, extra = waits[-cap:], waits[:-cap]
                for k, w in enumerate(extra):
                    n += 1
                    nop = mybir.InstNoOp(name=f"wsplit_{n}", ins=[], outs=[])
                    nop.engine = inst.engine
                    nop.sync_info = bass_rust.SyncInfo(on_wait=[w], on_update=[])
                    insts.insert(i + k, nop)
                inst.sync_info = bass_rust.SyncInfo(
                    on_wait=keep, on_update=list(si.on_update)
                )
                i += len(extra) + 1


# ---------------------------------------------------------------- device IR
def build_nc(n_steps=S, split_waits=True, repeat=1, stages=("dma", "compute")):
    groups = (n_steps + 3) // 4
    nc = bass.Bass("TRN2", target_bir_lowering=False, debug=False, num_devices=CORES)

    def din(name, shape, dt=BF16):
        return nc.dram_tensor(name, list(shape), dt, kind="ExternalInput").ap()

    qT_d = din("qT", (n_steps, D, Q))
    kvT_d = din("kvT", (groups, D, 4 * K))
    wq_d = din("wq", (D, D))
    wk_d = din("wk", (D, D))
    wv_d = din("wv", (D, D))
    wo_d = din("wo", (D, D))
    bqs_d = din("bqs", (D,), F32)
    bk_d = din("bk", (D,), F32)
    bo_d = din("bob", (P, D), F32)
    eb_d = din("EB", (K, Q))
    sel0_d = din("sel0", (K, 64))
    vzero_d = din("vzero", (P, 4, 2, P))
    out_d = nc.dram_tensor("out", [n_steps, Q, D], F32, kind="ExternalOutput").ap()

    from contextlib import ExitStack

    with tile.TileContext(nc) as tc, nc.allow_low_precision(
        reason="bf16 matmuls: tolerance is 2e-2, bf16 lands ~5e-3"
    ), ExitStack() as stack:
        ec = stack.enter_context
        cst = ec(tc.tile_pool(name="cst", bufs=1))
        qin_p = ec(tc.tile_pool(name="qin", bufs=2))
        kv_p = ec(tc.tile_pool(name="kv", bufs=2))
        qt_p = ec(tc.tile_pool(name="qt", bufs=8))
        kt_p = ec(tc.tile_pool(name="kt", bufs=8))
        ex_p = ec(tc.tile_pool(name="ex", bufs=6))
        e2_p = ec(tc.tile_pool(name="e2", bufs=10))
        rb_p = ec(tc.tile_pool(name="rb", bufs=4))
        ot_p = ec(tc.tile_pool(name="ot", bufs=8))
        oo_p = ec(tc.tile_pool(name="oo", bufs=3))
        pp = ec(tc.tile_pool(name="pp", bufs=2, space="PSUM"))
        pl = ec(tc.tile_pool(name="pl", bufs=2, space="PSUM"))
        po = ec(tc.tile_pool(name="po", bufs=2, space="PSUM"))
        pm = ec(tc.tile_pool(name="pm", bufs=2, space="PSUM"))
        if True:
            # ---- constants
            wq_sb = cst.tile([P, 4, D], BF16)
            wk_sb = cst.tile([P, 4, D], BF16)
            wv_sb = cst.tile([P, 4, D], BF16)
            wo_sb = cst.tile([P, 4, D], BF16)
            for w_sb, w_d in ((wq_sb, wq_d), (wk_sb, wk_d), (wv_sb, wv_d), (wo_sb, wo_d)):
                nc.sync.dma_start(w_sb, w_d.rearrange("(ct p) d -> p ct d", p=P))
            bqs_sb = cst.tile([P, 4], F32)
            bk_sb = cst.tile([P, 4], F32)
            nc.sync.dma_start(bqs_sb, bqs_d.rearrange("(dt p) -> p dt", p=P))
            nc.sync.dma_start(bk_sb, bk_d.rearrange("(dt p) -> p dt", p=P))
            bo_sb = cst.tile([P, D], F32)
            nc.sync.dma_start(bo_sb, bo_d)
            eb_sb = cst.tile([K, Q], BF16)
            nc.sync.dma_start(eb_sb, eb_d)
            sel0_sb = cst.tile([K, 64], BF16)
            nc.sync.dma_start(sel0_sb, sel0_d)
            sel1_sb = sel0_sb
            # persistent double-buffered zero-padded v tiles (zero halves DMA'd
            # once; per-step DVE copies only touch the v halves)
            vp_bufs = []
            for i in range(2):
                vb = cst.tile([P, 4, 2, P], BF16, tag=f"vp{i}")
                nc.sync.dma_start(vb, vzero_d)
                vp_bufs.append(vb)

            from contextlib import nullcontext

            loop_cm = tc.For_i(0, repeat, 1) if repeat > 1 else nullcontext()
            with loop_cm:
                body_steps(
                    nc, n_steps,
                    kv_p, qin_p, qt_p, kt_p, ex_p, e2_p, rb_p, ot_p, oo_p,
                    pp, pl, po, pm,
                    wq_sb, wk_sb, wv_sb, wo_sb, bqs_sb, bk_sb, bo_sb,
                    eb_sb, sel0_sb, sel1_sb, vp_bufs,
                    qT_d, kvT_d, out_d, stages,
                )

    if split_waits:
        _split_waits(nc)
    return nc


def body_steps(
    nc, n_steps,
    kv_p, qin_p, qt_p, kt_p, ex_p, e2_p, rb_p, ot_p, oo_p,
    pp, pl, po, pm,
    wq_sb, wk_sb, wv_sb, wo_sb, bqs_sb, bk_sb, bo_sb,
    eb_sb, sel0_sb, sel1_sb, vp_bufs,
    qT_d, kvT_d, out_d, stages=("dma", "compute"),
):
    do_dma = "dma" in stages
    do_compute = "compute" in stages
    groups = (n_steps + 3) // 4

    if not do_compute:
        # DMA-only stage for bisect timing
        for s in range(n_steps):
            g, s4 = divmod(s, 4)
            if do_dma:
                if s4 == 0:
                    kv_sb = kv_p.tile([P, 4, 4 * K], BF16, tag="kv")
                    nc.sync.dma_start(
                        kv_sb, kvT_d[g].rearrange("(ct p) x -> p ct x", p=P)
                    )
                qin_sb = qin_p.tile([P, 4, Q], BF16, tag="qin")
                nc.sync.dma_start(
                    qin_sb, qT_d[s].rearrange("(ct p) i -> p ct i", p=P)
                )
                oo_t = oo_p.tile([P, 4, D], F32, tag="oo")
                nc.vector.tensor_copy(oo_t[:, 0:2, :], qin_sb[:, 0:2, :])
                nc.vector.tensor_copy(oo_t[:, 2:4, :], qin_sb[:, 2:4, :])
                nc.sync.dma_start(
                    out_d[s].rearrange("(it p) d -> p it d", p=P), oo_t
                )
        return

    # ---------------- pipeline state
    kv_tiles = {}
    qin_tiles = {}
    qts = {}   # step -> [qt tile per dt]
    kts = {}   # group -> [kt tile per dt]
    e2s = {}   # step -> [e2 tile per head]
    ots = {}   # step -> [ot tile per pair]
    oos = {}   # step -> oo tile

    def dma_kv(g):
        if not do_dma or g >= groups or g in kv_tiles:
            return
        t = kv_p.tile([P, 4, 4 * K], BF16, tag="kv")
        nc.sync.dma_start(t, kvT_d[g].rearrange("(ct p) x -> p ct x", p=P))
        kv_tiles[g] = t

    def dma_qin(s):
        if not do_dma or s >= n_steps or s in qin_tiles:
            return
        t = qin_p.tile([P, 4, Q], BF16, tag="qin")
        nc.sync.dma_start(t, qT_d[s].rearrange("(ct p) i -> p ct i", p=P))
        qin_tiles[s] = t

    def kproj_dt(g, dt):
        kv_sb = kv_tiles[g] if do_dma else wk_sb
        ps_k = pp.tile([P, Q], F32, tag="pp")
        for ct in range(4):
            nc.tensor.matmul(
                ps_k,
                wk_sb[:, ct, dt * P : (dt + 1) * P],
                kv_sb[:, ct, :],
                start=(ct == 0),
                stop=(ct == 3),
            )
        kt_t = kt_p.tile([P, 4, K], BF16, tag="kt")
        nc.scalar.activation(
            kt_t.rearrange("p s j -> p (s j)"), ps_k, ACT_IDENT,
            bias=bk_sb[:, dt : dt + 1],
        )
        kts.setdefault(g, []).append(kt_t)

    def qproj_dt(s, dt):
        qin_sb = qin_tiles[s] if do_dma else wq_sb
        ps_q = pp.tile([P, Q], F32, tag="pp")
        for ct in range(4):
            nc.tensor.matmul(
                ps_q,
                wq_sb[:, ct, dt * P : (dt + 1) * P],
                qin_sb[:, ct, :],
                start=(ct == 0),
                stop=(ct == 3),
            )
        qt_t = qt_p.tile([P, Q], BF16, tag="qt")
        nc.scalar.activation(
            qt_t, ps_q, ACT_IDENT, bias=bqs_sb[:, dt : dt + 1], scale=SCALE
        )
        qts.setdefault(s, []).append(qt_t)

    def vproj(s):
        g, s4 = divmod(s, 4)
        kv_sb = kv_tiles[g] if do_dma else wk_sb
        ps_v = pp.tile([P, D], F32, tag="pp")
        for ct in range(4):
            nc.tensor.matmul(
                ps_v,
                kv_sb[:, ct, s4 * K : (s4 + 1) * K],
                wv_sb[:, ct, :],
                start=(ct == 0),
                stop=(ct == 3),
            )
        # zero-padded per-head-pair lhsT tiles [j, dt, parity, 128]: even head
        # occupies columns 0:64, odd head 64:128, so the AV matmul pair lands
        # both heads in one PSUM bank at partition base 0.
        vp = vp_bufs[s % 2]
        psv4 = ps_v.rearrange("p (a b c) -> p a b c", b=2, c=HD)
        nc.vector.tensor_copy(vp[:, :, 0, 0:64], psv4[:, :, 0, :])
        nc.vector.tensor_copy(vp[:, :, 1, 64:128], psv4[:, :, 1, :])

    def proj_chunks(s):
        """Projection work for step s, split into filler-sized chunks."""
        if s >= n_steps:
            return []
        g, s4 = divmod(s, 4)
        out = []
        if s4 == 0:
            out.append(lambda: (kproj_dt(g, 0), kproj_dt(g, 1)))
            out.append(lambda: (kproj_dt(g, 2), kproj_dt(g, 3)))
        out.append(lambda: (qproj_dt(s, 0), qproj_dt(s, 1)))
        out.append(lambda: (qproj_dt(s, 2), qproj_dt(s, 3)))
        out.append(lambda: vproj(s))
        return out

    def qk(s, h):
        """One head's logits matmul + exp (ACT) + positional-bias multiply
        (DVE, precomputed EB table; also applies key_mask)."""
        g, s4 = divmod(s, 4)
        hb = 64 * (h % 2)
        dt = h // 2
        ps_l = pl.tile([P, Q], F32, tag="pl")
        nc.tensor.matmul(
            ps_l,
            kts[g][dt][hb : hb + 64, s4, :],
            qts[s][dt][hb : hb + 64, :],
            start=True,
            stop=True,
        )
        ex_t = ex_p.tile([P, Q], BF16, tag="ex")
        nc.scalar.activation(ex_t, ps_l, ACT_EXP)
        e2_t = e2_p.tile([P, Q], BF16, tag="e2")
        nc.vector.tensor_tensor(e2_t, ex_t, eb_sb, MUL)
        e2s.setdefault(s, []).append(e2_t)

    def pair(s, p):
        """AV matmuls + softmax-sum matmuls + normalize for head pair p."""
        vp = vp_bufs[s % 2]
        ps_pair = po.tile([P, Q], F32, tag="po")
        ps_sum = pm.tile([P, Q], F32, tag="pm")
        for par in range(2):
            nc.tensor.matmul(
                ps_pair, vp[:, p, par, :], e2s[s][2 * p + par],
                start=(par == 0), stop=(par == 1),
            )
        # softmax sums: 64-col ones weights into the two halves of one bank;
        # adjacent col-group matmuls can run concurrently in the PE array
        for par in range(2):
            nc.tensor.matmul(
                ps_sum[64 * par : 64 * par + 64, :], sel0_sb, e2s[s][2 * p + par],
                start=True, stop=True, tile_position=(0, 64 * par),
                skip_group_check=True,
            )
        rb_t = rb_p.tile([P, Q], F32, tag="rb")
        nc.vector.reciprocal(rb_t, ps_sum)
        ot_t = ot_p.tile([P, Q], BF16, tag="ot")
        nc.vector.tensor_tensor(ot_t, ps_pair, rb_t, MUL)
        ots.setdefault(s, []).append(ot_t)

    def oproj_it(s, it):
        if s < 0 or s >= n_steps:
            return
        if s not in oos:
            oos[s] = oo_p.tile([P, 4, D], F32, tag="oo", name=f"oo_{s}")
        ps_f = pp.tile([P, D], F32, tag="pp")
        for dt in range(4):
            nc.tensor.matmul(
                ps_f,
                ots[s][dt][:, it * P : (it + 1) * P],
                wo_sb[:, dt, :],
                start=(dt == 0),
                stop=(dt == 3),
            )
        nc.vector.tensor_tensor(oos[s][:, it, :], ps_f, bo_sb, ADD)

    def dma_out(s):
        if s < 0 or s >= n_steps or not do_dma:
            return
        nc.sync.dma_start(
            out_d[s].rearrange("(it p) d -> p it d", p=P), oos[s]
        )

    # ---------------- pipelined emission
    # step s: QK section (o-proj of s-1 interleaved between QK head pairs),
    # then AV pairs (projections of s+1 interleaved between pairs).
    dma_kv(0)
    dma_qin(0)
    for c in proj_chunks(0):
        c()
    for s in range(n_steps):
        g, s4 = divmod(s, 4)
        # input prefetch ~1 step ahead of the projections that consume them
        dma_qin(s + 1)
        if s4 == 1:
            dma_kv(g + 1)
        qk(s, 0)
        qk(s, 1)
        for blk in range(3):
            if s > 0:
                oproj_it(s - 1, blk)
            qk(s, 2 + 2 * blk)
            qk(s, 3 + 2 * blk)
        if s > 0:
            oproj_it(s - 1, 3)
            dma_out(s - 1)
        nxt = proj_chunks(s + 1)
        pair(s, 0)
        pair(s, 1)
        if nxt:
            nxt[0]()
        pair(s, 2)
        if len(nxt) > 1:
            nxt[1]()
        pair(s, 3)
        for c in nxt[2:]:
            c()
        # drop refs no longer needed
        e2s.pop(s, None)
        if s >= 2:
            qts.pop(s - 2, None)
            ots.pop(s - 2, None)
            oos.pop(s - 2, None)
    for it in range(4):
        oproj_it(n_steps - 1, it)
    dma_out(n_steps - 1)


# ---------------------------------------------------------------- host prep
def make_in_maps(inputs, n_steps=S, cores=CORES):
    q_in = np.ascontiguousarray(np.asarray(inputs["query"], dtype=np.float32))
    kv_in = np.ascontiguousarray(np.asarray(inputs["key_value"], dtype=np.float32))
    qp = np.asarray(inputs["query_pos"], dtype=np.float32)
    kp = np.asarray(inputs["key_pos"], dtype=np.float32)
    mask = np.asarray(inputs["key_mask"])
    Wq = np.asarray(inputs["Wq"], dtype=np.float32)
    Wk = np.asarray(inputs["Wk"], dtype=np.float32)
    Wv = np.asarray(inputs["Wv"], dtype=np.float32)
    Wo = np.asarray(inputs["Wo"], dtype=np.float32)
    bq = np.asarray(inputs["bq"], dtype=np.float32)
    bk = np.asarray(inputs["bk"], dtype=np.float32)
    bv = np.asarray(inputs["bv"], dtype=np.float32)
    bo = np.asarray(inputs["bo"], dtype=np.float32)
    Wqb = np.asarray(inputs["Wqb"], dtype=np.float32)
    Wkb = np.asarray(inputs["Wkb"], dtype=np.float32)

    groups = (n_steps + 3) // 4
    bo2 = (
        bo.astype(np.float64) + Wo.astype(np.float64) @ bv.astype(np.float64)
    ).astype(np.float32)
    shared = {
        "wq": np.ascontiguousarray(Wq.T).astype(NPBF16),
        "wk": np.ascontiguousarray(Wk.T).astype(NPBF16),
        "wv": np.ascontiguousarray(Wv.T).astype(NPBF16),
        "wo": np.ascontiguousarray(Wo.T).astype(NPBF16),
        "bqs": (bq * SCALE).astype(np.float32),
        "bk": bk,
        "bob": np.broadcast_to(bo2, (P, D)).copy(),
        "vzero": np.zeros((P, 4, 2, P), NPBF16),
        "sel0": np.ones((K, 64), NPBF16),
    }

    # EB[j, i] = exp(dist_bias + lead_bias) with key_mask folded in (fp64 on
    # the host, O(B*K*Q) tiny)
    eb_per_b = {}
    for b in range(B):
        kp64 = kp[b].astype(np.float64)
        qp64 = qp[b].astype(np.float64)
        d2 = ((kp64[:, None, :] - qp64[None, :, :]) ** 2).sum(-1)  # [K, Q]
        lead = (
            (kp64 @ Wkb.T.astype(np.float64)) @ (qp64 @ Wqb.T.astype(np.float64)).T
        ) / np.sqrt(float(LOW_RANK))
        bias = -d2 * SIG + lead
        eb = np.exp(bias)
        eb[~mask[b], :] = 0.0
        eb_per_b[b] = eb.astype(np.float32).astype(NPBF16)

    steps_per_b = N  # 32
    in_maps = []
    for c in range(cores):
        start = c * n_steps
        b = start // steps_per_b
        n0 = start % steps_per_b
        qT = np.ascontiguousarray(
            q_in[b, n0 : n0 + n_steps].transpose(0, 2, 1)
        ).astype(NPBF16)  # [S, D, Q]
        kvT = np.ascontiguousarray(
            kv_in[b, n0 : n0 + n_steps]
            .reshape(groups, 4, K, D)
            .transpose(0, 3, 1, 2)
            .reshape(groups, D, 4 * K)
        ).astype(NPBF16)
        in_maps.append({"qT": qT, "kvT": kvT, "EB": eb_per_b[b], **shared})
    return in_maps


_NC_CACHE = {}


def kernel(**inputs) -> np.ndarray:
    if S not in _NC_CACHE:
        _NC_CACHE[S] = build_nc(S)
    nc = _NC_CACHE[S]
    in_maps = make_in_maps(inputs)
    res = run_bass_kernel_spmd(nc, in_maps, core_ids=list(range(CORES)))
    out = np.empty((B, N, Q, D), np.float32)
    for c in range(CORES):
        start = c * S
        b = start // N
        n0 = start % N
        out[b, n0 : n0 + S] = res.results[c]["out"]
    return out


# revision 22
# speedup vs baseline: 1.0245x; 1.0029x over previous
"""Trainium2 Bass kernel for nn_BackgroundFirstSourceFieldEEG (dense attention
with Gaussian-distance + low-rank leadfield softmax bias).

Strategy
--------
Data-parallel over batch*n_steps: the 4*32 = 128 (b, n) attention steps are
split 16-per-core across 8 NeuronCores (each core lands inside a single b, so
positional-bias factors are per-core constants).

All matmuls run in bf16 (fp32 PSUM accumulation). Measured on this part, a
back-to-back N=512 matmul stream sustains ~280-315 ns/MM regardless of
f32r/bf16 (self-loading weight stream dominates the gap to the 213 ns
roofline), so the win over the f32r baseline comes from cutting the MM count
per step from 76 to 64:

  - The softmax positional bias (Gaussian distance + low-rank leadfield) is
    precomputed on the host as EB = exp(bias) [K, Q] fp32->bf16 (constant per
    core, key_mask folded in as EB=0) and applied as one DVE multiply on the
    exp'd logits per head - this removes the 8 per-head bias matmuls and the
    whole Dekker-split machinery the f32r version needed.
  - bo (+ Wo @ bv) is added via the DVE copy-back against a host-broadcast
    [128, D] table - removes 4 ones-row matmuls per step.

Per step: 16 q-proj + 4 k-proj (amortized) + 4 v-proj + 8 QK + 8 AV(pair)
+ 8 sel-sum + 16 o-proj = 64 MMs.

Activations stay transposed (feature dim on partitions) end-to-end:
  qT[d,i] = WqT.T @ queryT  (bias+scale fused in the ACT copy-back)
  kT[d,j] = WkT.T @ kvT     (+bk, batched 4 steps -> 512-wide)
  v[j,d]  = kvT.T @ WvT     (bv folded into bo on host: sum_j attn = 1)
  logitsT[j,i] = kT_h.T @ qT_h      (64-row contraction per head)
  ex = exp(logitsT) * EB            (ACT exp, DVE mul)
  oT[d,i] = v_h.T @ ex_h            (head pairs packed into one PSUM bank)
  sums via select-matrix matmuls, reciprocal+normalize on DVE
  out[i,do] = oT.T @ WoT            (+bo via DVE add of broadcast table)
"""

import sys

for _p in ("/opt/trn_rl_repo", "/root/.axon_site/_ro/trn_rl_repo"):
    if _p not in sys.path:
        sys.path.insert(0, _p)

import numpy as np

import bass_rust
import concourse.bass as bass
import concourse.mybir as mybir
import concourse.tile as tile
from concourse.bass_utils import run_bass_kernel_spmd

F32 = mybir.dt.float32
BF16 = mybir.dt.bfloat16
NPBF16 = mybir.dt.np(BF16)
ACT_IDENT = mybir.ActivationFunctionType.Identity
ACT_EXP = mybir.ActivationFunctionType.Exp
ACT_RECIP = mybir.ActivationFunctionType.Reciprocal
MUL = mybir.AluOpType.mult
ADD = mybir.AluOpType.add

B, N, Q, K, D = 4, 32, 512, 128, 512
H, HD = 8, 64
LOW_RANK = 8
SCALE = HD ** -0.5
SIGMA = 0.05
SIG = 1.0 / (2.0 * max(SIGMA * SIGMA, 1e-6))
CORES = 8
S = (B * N) // CORES  # steps per core
P = 128


# ---------------------------------------------------------------- wait split
def _split_waits(nc, cap_mm=1, cap_other=1):
    """walrus in this container rejects instructions with more than ~1 sync
    wait (self-loading matmuls) / few (ctrl). Move excess waits onto
    same-engine NoOps inserted right before the instruction."""
    n = 0
    for fn in nc.m.functions:
        for bb in fn.blocks:
            insts = bb.instructions  # live list
            i = 0
            while i < len(insts):
                inst = insts[i]
                si = inst.sync_info
                if si is None:
                    i += 1
                    continue
                cap = cap_mm if isinstance(inst, mybir.InstMatmult) else cap_other
                waits = list(si.on_wait)
                if len(waits) <= cap:
                    i += 1
                    continue
                keep, extra = waits[-cap:], waits[:-cap]
                for k, w in enumerate(extra):
                    n += 1
                    nop = mybir.InstNoOp(name=f"wsplit_{n}", ins=[], outs=[])
                    nop.engine = inst.engine
                    nop.sync_info = bass_rust.SyncInfo(on_wait=[w], on_update=[])
                    insts.insert(i + k, nop)
                inst.sync_info = bass_rust.SyncInfo(
                    on_wait=keep, on_update=list(si.on_update)
                )
                i += len(extra) + 1


# ---------------------------------------------------------------- device IR
def build_nc(n_steps=S, split_waits=True, repeat=1, stages=("dma", "compute")):
    groups = (n_steps + 3) // 4
    nc = bass.Bass("TRN2", target_bir_lowering=False, debug=False, num_devices=CORES)

    def din(name, shape, dt=BF16):
        return nc.dram_tensor(name, list(shape), dt, kind="ExternalInput").ap()

    qT_d = din("qT", (n_steps, D, Q))
    kvT_d = din("kvT", (groups, D, 4 * K))
    wq_d = din("wq", (D, D))
    wk_d = din("wk", (D, D))
    wv_d = din("wv", (D, D))
    wo_d = din("wo", (D, D))
    bqs_d = din("bqs", (D,), F32)
    bk_d = din("bk", (D,), F32)
    bo_d = din("bob", (P, D), F32)
    eb_d = din("EB", (K, Q))
    sel0_d = din("sel0", (K, 64))
    vzero_d = din("vzero", (P, 4, 2, P))
    out_d = nc.dram_tensor("out", [n_steps, Q, D], F32, kind="ExternalOutput").ap()

    from contextlib import ExitStack

    with tile.TileContext(nc) as tc, nc.allow_low_precision(
        reason="bf16 matmuls: tolerance is 2e-2, bf16 lands ~5e-3"
    ), ExitStack() as stack:
        ec = stack.enter_context
        cst = ec(tc.tile_pool(name="cst", bufs=1))
        qin_p = ec(tc.tile_pool(name="qin", bufs=2))
        kv_p = ec(tc.tile_pool(name="kv", bufs=2))
        qt_p = ec(tc.tile_pool(name="qt", bufs=8))
        kt_p = ec(tc.tile_pool(name="kt", bufs=8))
        ex_p = ec(tc.tile_pool(name="ex", bufs=6))
        e2_p = ec(tc.tile_pool(name="e2", bufs=10))
        rb_p = ec(tc.tile_pool(name="rb", bufs=4))
        ot_p = ec(tc.tile_pool(name="ot", bufs=8))
        oo_p = ec(tc.tile_pool(name="oo", bufs=3))
        pp = ec(tc.tile_pool(name="pp", bufs=2, space="PSUM"))
        pl = ec(tc.tile_pool(name="pl", bufs=2, space="PSUM"))
        po = ec(tc.tile_pool(name="po", bufs=2, space="PSUM"))
        pm = ec(tc.tile_pool(name="pm", bufs=2, space="PSUM"))
        if True:
            # ---- constants
            wq_sb = cst.tile([P, 4, D], BF16)
            wk_sb = cst.tile([P, 4, D], BF16)
            wv_sb = cst.tile([P, 4, D], BF16)
            wo_sb = cst.tile([P, 4, D], BF16)
            for w_sb, w_d in ((wq_sb, wq_d), (wk_sb, wk_d), (wv_sb, wv_d), (wo_sb, wo_d)):
                nc.sync.dma_start(w_sb, w_d.rearrange("(ct p) d -> p ct d", p=P))
            bqs_sb = cst.tile([P, 4], F32)
            bk_sb = cst.tile([P, 4], F32)
            nc.sync.dma_start(bqs_sb, bqs_d.rearrange("(dt p) -> p dt", p=P))
            nc.sync.dma_start(bk_sb, bk_d.rearrange("(dt p) -> p dt", p=P))
            bo_sb = cst.tile([P, D], F32)
            nc.sync.dma_start(bo_sb, bo_d)
            eb_sb = cst.tile([K, Q], BF16)
            nc.sync.dma_start(eb_sb, eb_d)
            sel0_sb = cst.tile([K, 64], BF16)
            nc.sync.dma_start(sel0_sb, sel0_d)
            sel1_sb = sel0_sb
            # persistent double-buffered zero-padded v tiles (zero halves DMA'd
            # once; per-step DVE copies only touch the v halves)
            vp_bufs = []
            for i in range(2):
                vb = cst.tile([P, 4, 2, P], BF16, tag=f"vp{i}")
                nc.sync.dma_start(vb, vzero_d)
                vp_bufs.append(vb)

            from contextlib import nullcontext

            loop_cm = tc.For_i(0, repeat, 1) if repeat > 1 else nullcontext()
            with loop_cm:
                body_steps(
                    nc, n_steps,
                    kv_p, qin_p, qt_p, kt_p, ex_p, e2_p, rb_p, ot_p, oo_p,
                    pp, pl, po, pm,
                    wq_sb, wk_sb, wv_sb, wo_sb, bqs_sb, bk_sb, bo_sb,
                    eb_sb, sel0_sb, sel1_sb, vp_bufs,
                    qT_d, kvT_d, out_d, stages,
                )

    if split_waits:
        _split_waits(nc)
    return nc


def body_steps(
    nc, n_steps,
    kv_p, qin_p, qt_p, kt_p, ex_p, e2_p, rb_p, ot_p, oo_p,
    pp, pl, po, pm,
    wq_sb, wk_sb, wv_sb, wo_sb, bqs_sb, bk_sb, bo_sb,
    eb_sb, sel0_sb, sel1_sb, vp_bufs,
    qT_d, kvT_d, out_d, stages=("dma", "compute"),
):
    do_dma = "dma" in stages
    do_compute = "compute" in stages
    groups = (n_steps + 3) // 4

    if not do_compute:
        # DMA-only stage for bisect timing
        for s in range(n_steps):
            g, s4 = divmod(s, 4)
            if do_dma:
                if s4 == 0:
                    kv_sb = kv_p.tile([P, 4, 4 * K], BF16, tag="kv")
                    nc.sync.dma_start(
                        kv_sb, kvT_d[g].rearrange("(ct p) x -> p ct x", p=P)
                    )
                qin_sb = qin_p.tile([P, 4, Q], BF16, tag="qin")
                nc.sync.dma_start(
                    qin_sb, qT_d[s].rearrange("(ct p) i -> p ct i", p=P)
                )
                oo_t = oo_p.tile([P, 4, D], F32, tag="oo")
                nc.vector.tensor_copy(oo_t[:, 0:2, :], qin_sb[:, 0:2, :])
                nc.vector.tensor_copy(oo_t[:, 2:4, :], qin_sb[:, 2:4, :])
                nc.sync.dma_start(
                    out_d[s].rearrange("(it p) d -> p it d", p=P), oo_t
                )
        return

    # ---------------- pipeline state
    kv_tiles = {}
    qin_tiles = {}
    qts = {}   # step -> [qt tile per dt]
    kts = {}   # group -> [kt tile per dt]
    e2s = {}   # step -> [e2 tile per head]
    ots = {}   # step -> [ot tile per pair]
    oos = {}   # step -> oo tile

    def dma_kv(g):
        if not do_dma or g >= groups or g in kv_tiles:
            return
        t = kv_p.tile([P, 4, 4 * K], BF16, tag="kv")
        nc.sync.dma_start(t, kvT_d[g].rearrange("(ct p) x -> p ct x", p=P))
        kv_tiles[g] = t

    def dma_qin(s):
        if not do_dma or s >= n_steps or s in qin_tiles:
            return
        t = qin_p.tile([P, 4, Q], BF16, tag="qin")
        nc.sync.dma_start(t, qT_d[s].rearrange("(ct p) i -> p ct i", p=P))
        qin_tiles[s] = t

    def kproj_dt(g, dt):
        kv_sb = kv_tiles[g] if do_dma else wk_sb
        ps_k = pp.tile([P, Q], F32, tag="pp")
        for ct in range(4):
            nc.tensor.matmul(
                ps_k,
                wk_sb[:, ct, dt * P : (dt + 1) * P],
                kv_sb[:, ct, :],
                start=(ct == 0),
                stop=(ct == 3),
            )
        kt_t = kt_p.tile([P, 4, K], BF16, tag="kt")
        nc.scalar.activation(
            kt_t.rearrange("p s j -> p (s j)"), ps_k, ACT_IDENT,
            bias=bk_sb[:, dt : dt + 1],
        )
        kts.setdefault(g, []).append(kt_t)

    def qproj_dt(s, dt):
        qin_sb = qin_tiles[s] if do_dma else wq_sb
        ps_q = pp.tile([P, Q], F32, tag="pp")
        for ct in range(4):
            nc.tensor.matmul(
                ps_q,
                wq_sb[:, ct, dt * P : (dt + 1) * P],
                qin_sb[:, ct, :],
                start=(ct == 0),
                stop=(ct == 3),
            )
        qt_t = qt_p.tile([P, Q], BF16, tag="qt")
        nc.scalar.activation(
            qt_t, ps_q, ACT_IDENT, bias=bqs_sb[:, dt : dt + 1], scale=SCALE
        )
        qts.setdefault(s, []).append(qt_t)

    def vproj(s):
        g, s4 = divmod(s, 4)
        kv_sb = kv_tiles[g] if do_dma else wk_sb
        ps_v = pp.tile([P, D], F32, tag="pp")
        for ct in range(4):
            nc.tensor.matmul(
                ps_v,
                kv_sb[:, ct, s4 * K : (s4 + 1) * K],
                wv_sb[:, ct, :],
                start=(ct == 0),
                stop=(ct == 3),
            )
        # zero-padded per-head-pair lhsT tiles [j, dt, parity, 128]: even head
        # occupies columns 0:64, odd head 64:128, so the AV matmul pair lands
        # both heads in one PSUM bank at partition base 0.
        vp = vp_bufs[s % 2]
        psv4 = ps_v.rearrange("p (a b c) -> p a b c", b=2, c=HD)
        nc.vector.tensor_copy(vp[:, :, 0, 0:64], psv4[:, :, 0, :])
        nc.vector.tensor_copy(vp[:, :, 1, 64:128], psv4[:, :, 1, :])

    def proj_chunks(s):
        """Projection work for step s, split into filler-sized chunks."""
        if s >= n_steps:
            return []
        g, s4 = divmod(s, 4)
        out = []
        if s4 == 0:
            out.append(lambda: (kproj_dt(g, 0), kproj_dt(g, 1)))
            out.append(lambda: (kproj_dt(g, 2), kproj_dt(g, 3)))
        out.append(lambda: (qproj_dt(s, 0), qproj_dt(s, 1)))
        out.append(lambda: (qproj_dt(s, 2), qproj_dt(s, 3)))
        out.append(lambda: vproj(s))
        return out

    def qk(s, h):
        """One head's logits matmul + exp (ACT) + positional-bias multiply
        (DVE, precomputed EB table; also applies key_mask)."""
        g, s4 = divmod(s, 4)
        hb = 64 * (h % 2)
        dt = h // 2
        ps_l = pl.tile([P, Q], F32, tag="pl")
        nc.tensor.matmul(
            ps_l,
            kts[g][dt][hb : hb + 64, s4, :],
            qts[s][dt][hb : hb + 64, :],
            start=True,
            stop=True,
        )
        ex_t = ex_p.tile([P, Q], BF16, tag="ex")
        nc.scalar.activation(ex_t, ps_l, ACT_EXP)
        e2_t = e2_p.tile([P, Q], BF16, tag="e2")
        nc.vector.tensor_tensor(e2_t, ex_t, eb_sb, MUL)
        e2s.setdefault(s, []).append(e2_t)

    def pair(s, p):
        """AV matmuls + softmax-sum matmuls + normalize for head pair p."""
        vp = vp_bufs[s % 2]
        ps_pair = po.tile([P, Q], F32, tag="po")
        ps_sum = pm.tile([P, Q], F32, tag="pm")
        for par in range(2):
            nc.tensor.matmul(
                ps_pair, vp[:, p, par, :], e2s[s][2 * p + par],
                start=(par == 0), stop=(par == 1),
            )
        # softmax sums: 64-col ones weights into the two halves of one bank;
        # adjacent col-group matmuls can run concurrently in the PE array
        for par in range(2):
            nc.tensor.matmul(
                ps_sum[64 * par : 64 * par + 64, :], sel0_sb, e2s[s][2 * p + par],
                start=True, stop=True, tile_position=(0, 64 * par),
                skip_group_check=True,
            )
        rb_t = rb_p.tile([P, Q], F32, tag="rb")
        nc.vector.reciprocal(rb_t, ps_sum)
        ot_t = ot_p.tile([P, Q], BF16, tag="ot")
        nc.vector.tensor_tensor(ot_t, ps_pair, rb_t, MUL)
        ots.setdefault(s, []).append(ot_t)

    def oproj_it(s, it):
        if s < 0 or s >= n_steps:
            return
        if s not in oos:
            oos[s] = oo_p.tile([P, 4, D], F32, tag="oo", name=f"oo_{s}")
        ps_f = pp.tile([P, D], F32, tag="pp")
        for dt in range(4):
            nc.tensor.matmul(
                ps_f,
                ots[s][dt][:, it * P : (it + 1) * P],
                wo_sb[:, dt, :],
                start=(dt == 0),
                stop=(dt == 3),
            )
        nc.vector.tensor_tensor(oos[s][:, it, :], ps_f, bo_sb, ADD)

    def dma_out(s):
        if s < 0 or s >= n_steps or not do_dma:
            return
        nc.sync.dma_start(
            out_d[s].rearrange("(it p) d -> p it d", p=P), oos[s]
        )

    # ---------------- pipelined emission
    # step s: QK section (o-proj of s-1 interleaved between QK head pairs),
    # then AV pairs (projections of s+1 interleaved between pairs).
    dma_kv(0)
    dma_qin(0)
    for c in proj_chunks(0):
        c()
    for s in range(n_steps):
        g, s4 = divmod(s, 4)
        # input prefetch ~1 step ahead of the projections that consume them
        dma_qin(s + 1)
        if s4 == 1:
            dma_kv(g + 1)
        qk(s, 0)
        qk(s, 1)
        for blk in range(3):
            if s > 0:
                oproj_it(s - 1, blk)
            qk(s, 2 + 2 * blk)
            qk(s, 3 + 2 * blk)
        if s > 0:
            oproj_it(s - 1, 3)
            dma_out(s - 1)
        nxt = proj_chunks(s + 1)
        pair(s, 0)
        pair(s, 1)
        if nxt:
            nxt[0]()
        pair(s, 2)
        if len(nxt) > 1:
            nxt[1]()
        pair(s, 3)
        for c in nxt[2:]:
            c()
        # drop refs no longer needed
        e2s.pop(s, None)
        if s >= 2:
            qts.pop(s - 2, None)
            ots.pop(s - 2, None)
            oos.pop(s - 2, None)
    for it in range(4):
        oproj_it(n_steps - 1, it)
    dma_out(n_steps - 1)


# ---------------------------------------------------------------- host prep
def make_in_maps(inputs, n_steps=S, cores=CORES):
    q_in = np.ascontiguousarray(np.asarray(inputs["query"], dtype=np.float32))
    kv_in = np.ascontiguousarray(np.asarray(inputs["key_value"], dtype=np.float32))
    qp = np.asarray(inputs["query_pos"], dtype=np.float32)
    kp = np.asarray(inputs["key_pos"], dtype=np.float32)
    mask = np.asarray(inputs["key_mask"])
    Wq = np.asarray(inputs["Wq"], dtype=np.float32)
    Wk = np.asarray(inputs["Wk"], dtype=np.float32)
    Wv = np.asarray(inputs["Wv"], dtype=np.float32)
    Wo = np.asarray(inputs["Wo"], dtype=np.float32)
    bq = np.asarray(inputs["bq"], dtype=np.float32)
    bk = np.asarray(inputs["bk"], dtype=np.float32)
    bv = np.asarray(inputs["bv"], dtype=np.float32)
    bo = np.asarray(inputs["bo"], dtype=np.float32)
    Wqb = np.asarray(inputs["Wqb"], dtype=np.float32)
    Wkb = np.asarray(inputs["Wkb"], dtype=np.float32)

    groups = (n_steps + 3) // 4
    bo2 = (
        bo.astype(np.float64) + Wo.astype(np.float64) @ bv.astype(np.float64)
    ).astype(np.float32)
    shared = {
        "wq": np.ascontiguousarray(Wq.T).astype(NPBF16),
        "wk": np.ascontiguousarray(Wk.T).astype(NPBF16),
        "wv": np.ascontiguousarray(Wv.T).astype(NPBF16),
        "wo": np.ascontiguousarray(Wo.T).astype(NPBF16),
        "bqs": (bq * SCALE).astype(np.float32),
        "bk": bk,
        "bob": np.broadcast_to(bo2, (P, D)).copy(),
        "vzero": np.zeros((P, 4, 2, P), NPBF16),
        "sel0": np.ones((K, 64), NPBF16),
    }

    # EB[j, i] = exp(dist_bias + lead_bias) with key_mask folded in (fp64 on
    # the host, O(B*K*Q) tiny)
    eb_per_b = {}
    for b in range(B):
        kp64 = kp[b].astype(np.float64)
        qp64 = qp[b].astype(np.float64)
        d2 = ((kp64[:, None, :] - qp64[None, :, :]) ** 2).sum(-1)  # [K, Q]
        lead = (
            (kp64 @ Wkb.T.astype(np.float64)) @ (qp64 @ Wqb.T.astype(np.float64)).T
        ) / np.sqrt(float(LOW_RANK))
        bias = -d2 * SIG + lead
        eb = np.exp(bias)
        eb[~mask[b], :] = 0.0
        eb_per_b[b] = eb.astype(np.float32).astype(NPBF16)

    steps_per_b = N  # 32
    in_maps = []
    for c in range(cores):
        start = c * n_steps
        b = start // steps_per_b
        n0 = start % steps_per_b
        qT = np.ascontiguousarray(
            q_in[b, n0 : n0 + n_steps].transpose(0, 2, 1)
        ).astype(NPBF16)  # [S, D, Q]
        kvT = np.ascontiguousarray(
            kv_in[b, n0 : n0 + n_steps]
            .reshape(groups, 4, K, D)
            .transpose(0, 3, 1, 2)
            .reshape(groups, D, 4 * K)
        ).astype(NPBF16)
        in_maps.append({"qT": qT, "kvT": kvT, "EB": eb_per_b[b], **shared})
    return in_maps


_NC_CACHE = {}


def kernel(**inputs) -> np.ndarray:
    if S not in _NC_CACHE:
        _NC_CACHE[S] = build_nc(S)
    nc = _NC_CACHE[S]
    in_maps = make_in_maps(inputs)
    res = run_bass_kernel_spmd(nc, in_maps, core_ids=list(range(CORES)))
    out = np.empty((B, N, Q, D), np.float32)
    for c in range(CORES):
        start = c * S
        b = start // N
        n0 = start % N
        out[b, n0 : n0 + S] = res.results[c]["out"]
    return out
